# revision 12
# baseline (speedup 1.0000x reference)
"""ArcFace head forward on 8 Trainium2 NeuronCores (Bass, raw blocks).

Math (per batch row b, class c in {0,1}):
    feat_n = feat / max(||feat||, 1e-12)
    W_n    = W / max(||W_row||, 1e-12)
    cos    = clip(feat_n . W_n[c], -1+1e-7, 1-1e-7)
    cos_m  = cos*cos(0.5) - sqrt(1-cos^2)*sin(0.5)   # == cos(arccos(cos)+0.5)
    out    = 64 * (cos if c != label[b] else cos_m)

Distribution: pure data parallel: feat/label sharded along batch over 8
cores, W replicated; forward only, so no collectives.

Per-core pipeline (shard = 16384 rows x 512 f32 = 32 MB):
  - GpSimd SWDGE streams feat in 16 blocks of [128, 4096], converting
    f32 -> bf16 in flight (SDMA cast); 16 KB-contiguous per partition.
    First AND last blocks are split into chunk-DMAs so compute can
    start/finish at pair granularity instead of block granularity.
  - TensorE: per PAIR of [128,512] sub-tiles, 4x PE-transposes of the
    f32 VIEW of fb (each f32 element = 2 packed bf16 -> half the
    transpose instructions; transpose mode is bit-exact routing), then
    per sub-tile 4 accumulating matmuls: stationary = strided bf16
    views of the packed feat^T pair, moving = normalized-W^T pair
    chunks -> both class dots in PSUM [128,2]
  - ScalarE: copies transposed pairs PSUM->SBUF as f32 (matmul
    stationary must come from SBUF) + 2-of-8 sub-tiles' row
    sum-of-squares (activation Square + accum_out)
  - VectorE: 6-of-8 sub-tiles' sum-of-squares (fused self-mult STT,
    gated on the LOAD sem directly, not on ScalarE), per-block dot
    harvest from PSUM, and the batched epilogue
Ring-reuse safety: the loader double-waits (vdone AND ablk) before
recycling an fb slot, so VectorE never has to wait for ScalarE.
All dot accumulators share ONE PSUM bank (PE start_tensor_calc marks a
whole 2KB zero region but reads of marked-unwritten entries return the
stale (correct) data -- the quarter-tile transposes already rely on
this); that frees banks for a 4-deep transpose ring.
Row mapping: batch row b = blk*1024 + p*8 + j lives on partition p,
accumulator column t = blk*8 + j. Host glue only shards/reorders.
"""

import sys
from contextlib import ExitStack

import numpy as np

for _p in ("/opt/trn_rl_repo",):
    if _p not in sys.path:
        sys.path.insert(0, _p)

import concourse.bass as bass
import concourse.mybir as mybir
from concourse.bass_utils import run_bass_kernel_spmd

B, D, C = 131072, 512, 2
NCORES = 8
BS = B // NCORES          # 16384 rows per core
SUB = 512                 # columns per compute sub-tile
SPB = 8                   # sub-tiles per block
BLK_COLS = SUB * SPB      # 4096 (1024 batch rows)
NBLK = BS // (128 * SPB)  # 16
T = BS // 128             # 128 accumulator columns
NB = 5                    # fb ring depth (1 MB bf16 each)
SQ_DVE = 6                # sub-tiles per block whose sumsq runs on VectorE
PAIRS = T // 2            # transpose/copy granularity: 2 sub-tiles per pair
PPB = SPB // 2            # pairs per block (4)
TR = 3                    # tpb/ftT ring depth
LAG = 2                   # dots trail transposes by LAG pairs (needs TR >= LAG+1)
DPR = NB                  # d_ps ring = fb ring

S_SCALE = 64.0
MARGIN = 0.5
EPS = 1e-7
NORM_EPS = 1e-12
COS_M = float(np.cos(MARGIN))
SIN_M = float(np.sin(MARGIN))

F32 = mybir.dt.float32
BF16 = mybir.dt.bfloat16


def build_nc():
    nc = bass.Bass()
    AF = mybir.ActivationFunctionType
    OP = mybir.AluOpType

    feat = nc.declare_dram_parameter("feat", [BS, D], F32, isOutput=False)
    wrep = nc.declare_dram_parameter("wrep", [128, C * D], F32, isOutput=False)
    wrepT = nc.declare_dram_parameter("wrepT", [128, 2 * 4], F32, isOutput=False)
    lab = nc.declare_dram_parameter("lab", [128, T], F32, isOutput=False)
    identf = nc.declare_dram_parameter("identf", [128, 128], F32, isOutput=False)
    out = nc.declare_dram_parameter("out", [128, C * T], F32, isOutput=True)

    # feat[blk*1024 + p*8 + j, d] -> view[blk, p, j*512+d] (16KB/partition)
    feat_v = feat[:].rearrange("(blk p j) d -> blk p (j d)", p=128, j=SPB)

    with ExitStack() as ctx:
        def sb(name, shape, dt):
            return ctx.enter_context(nc.sbuf_tensor(name, shape, dt))

        def psum(name, shape, dt):
            return ctx.enter_context(nc.psum_tensor(name, shape, dt))

        def sem(name):
            return ctx.enter_context(nc.semaphore(name))

        w_f = sb("w_f", [128, C * D], F32)
        wrepT_sb = sb("wrepT_sb", [128, 8], F32)
        w_bfT = sb("w_bfT", [128, 8], BF16)     # [p, c*4+k] = WnT chunk layout
        lab_t = sb("lab_t", [128, T], F32)
        identf_sb = sb("identf_sb", [128, 128], F32)
        ss = sb("ss", [128, T], F32)
        dd = sb("dd", [128, C * T], F32)        # d0 | d1
        fbs = [sb(f"fb{k}", [128, BLK_COLS], BF16) for k in range(NB)]
        ftT = [sb(f"ftT{k}", [128, SUB], F32) for k in range(TR)]
        # scratch (dummy) outputs alternate buffers so consecutive same-engine
        # ops carry no WAW hazard (writes drain async; scratch is never read)
        sq_scr = [sb(f"sq_scr{k}", [128, SUB], F32) for k in range(2)]
        tt_scr = [sb(f"tt_scr{k}", [128, SUB], BF16) for k in range(2)]
        wss = sb("wss", [128, C], F32)
        wnorm = sb("wnorm", [128, C], F32)
        winv = sb("winv", [128, C], F32)
        norm = sb("norm", [128, T], F32)
        inv = sb("inv", [128, T], F32)
        cos0 = sb("cos0", [128, T], F32)
        cos1 = sb("cos1", [128, T], F32)
        sq2a = sb("sq2a", [128, T], F32)
        sq2b = sb("sq2b", [128, T], F32)
        sin0 = sb("sin0", [128, T], F32)
        sin1 = sb("sin1", [128, T], F32)
        tmp1 = sb("tmp1", [128, T], F32)
        tmp2 = sb("tmp2", [128, T], F32)
        oh64_0 = sb("oh64_0", [128, T], F32)    # 64*(1-lab)
        oh64_1 = sb("oh64_1", [128, T], F32)    # 64*lab
        out_stage = sb("out_stage", [128, C * T], F32)

        tpb = [psum(f"tpb{k}", [128, SUB], F32) for k in range(TR)]
        d_ps = [psum(f"dps{k}", [128, 2 * SPB], F32) for k in range(DPR)]

        pre = sem("pre")       # preamble DMAs: wrepT(16), w_f(32), lab(48)
        preI = sem("preI")     # identf DMA
        # One load-sem per fb ring slot: at most one DMA is ever outstanding
        # per sem (the ring gating guarantees it), so thresholds are exact
        # even under per-SDMA-engine completion skew.
        lds = [sem(f"ld{k}") for k in range(NB)]
        # blocks 0 and 15 are split into chunk-DMAs (one per pair) to hide
        # full-block latency at kernel start and end; ld0[c] hits 16 for
        # block 0's chunk c and 32 for block 15's chunk c.
        ld0 = [sem(f"ld0_{k}") for k in range(PPB)]
        ld0x = sem("ld0x")     # first 256KB half of block0 chunk0
        petr = sem("petr")     # PE transposes done for pair u -> u+1
        actcopy = sem("actcopy")  # ACT copy of pair u done -> u+1
        pedot = sem("pedot")   # PE dots done for pair u -> u+1
        ablk = sem("ablk")     # ACT done with block i -> i+1 (loader gating)
        vdone = sem("vdone")   # DVE done with block i -> i+1 (loader gating)
        ssdone = sem("ssdone")  # all DVE sumsq columns written
        sepi = sem("sepi")     # scalar milestones
        vepi = sem("vepi")     # vector milestones
        outd = sem("outd")

        # strided view of w_bfT: [p, g, c] with c-stride 4; g = qp*2+par
        w_bfT_g = w_bfT[:].rearrange("p (c g) -> p g c", g=4)

        # per-slot lds threshold for whole-block loads (blocks 1..14):
        # slot s receives incs from blocks s, s+NB, s+2NB (minus 0 and 15)
        def lds_thr(i):
            return i // NB + (1 if i % NB != 0 else 0)

        with nc.Block() as block:

            @block.sync
            def _(sync):
                sync.dma_start(out=identf_sb[:], in_=identf[:]).then_inc(preI, 16)
                sync.dma_start(out=wrepT_sb[:], in_=wrepT[:]).then_inc(pre, 16)
                sync.dma_start(out=w_f[:], in_=wrep[:]).then_inc(pre, 16)
                sync.dma_start(out=lab_t[:], in_=lab[:]).then_inc(pre, 16)
                sync.wait_ge(vepi, 4)
                sync.dma_start(
                    out=out[:, 0:T], in_=out_stage[:, 0:T]
                ).then_inc(outd, 16)
                sync.wait_ge(vepi, 5)
                sync.dma_start(
                    out=out[:, T:2 * T], in_=out_stage[:, T:2 * T]
                ).then_inc(outd, 16)
                sync.wait_ge(outd, 32)

            @block.gpsimd
            def _(gpsimd):
                gpsimd.dma_start(
                    out=fbs[0][:, 0:512], in_=feat_v[0][:, 0:512]
                ).then_inc(ld0x, 16)
                gpsimd.dma_start(
                    out=fbs[0][:, 512:1024], in_=feat_v[0][:, 512:1024]
                ).then_inc(ld0[0], 16)
                for c in range(1, PPB):
                    gpsimd.dma_start(
                        out=fbs[0][:, c * 1024:(c + 1) * 1024],
                        in_=feat_v[0][:, c * 1024:(c + 1) * 1024],
                    ).then_inc(ld0[c], 16)
                for i in range(1, NBLK):
                    if i >= NB:
                        # fb slot recycle: DVE iter i-NB+1 (sumsq of i-NB+1,
                        # lazy harvest of i-NB) AND ACT's sumsq of block i-NB
                        # must be done before slot i%NB is overwritten
                        gpsimd.wait_ge(vdone, i - NB + 2)
                        gpsimd.wait_ge(ablk, i - NB + 1)
                    if i == NBLK - 1:
                        # last block chunked so the tail drains at pair
                        # granularity (mirrors block 0)
                        for c in range(PPB):
                            gpsimd.dma_start(
                                out=fbs[i % NB][:, c * 1024:(c + 1) * 1024],
                                in_=feat_v[i][:, c * 1024:(c + 1) * 1024],
                            ).then_inc(ld0[c], 16)
                    else:
                        # converting DMA: f32 DRAM -> bf16 SBUF
                        gpsimd.dma_start(
                            out=fbs[i % NB][:], in_=feat_v[i]
                        ).then_inc(lds[i % NB], 16)

            @block.tensor
            def _(tensor):
                tensor.wait_ge(preI, 16)  # identf_sb ready
                # software-pipelined: transposes run LAG pairs ahead of dots.
                # Transposes operate on the f32 view of fb (bf16 pairs packed
                # per element): 4 transposes move a whole pair (2 sub-tiles).
                for u in range(PAIRS + LAG):
                    if u < PAIRS:
                        i = u // PPB
                        if i == 0:
                            if u == 0:
                                tensor.wait_ge(ld0x, 16)
                            else:
                                tensor.wait_ge(ld0[u], 16)
                        elif i == NBLK - 1:
                            tensor.wait_ge(ld0[u % PPB], 32)
                        elif u % PPB == 0:
                            tensor.wait_ge(lds[i % NB], 16 * lds_thr(i))
                        fb32 = fbs[i % NB][:].bitcast(F32)  # [128, 2048]
                        base = (u % PPB) * SUB              # f32 cols per pair
                        for q in range(4):
                            if u == 0 and q == 2:
                                tensor.wait_ge(ld0[0], 16)
                            ins = tensor.transpose(
                                tpb[u % TR][:, q * 128:(q + 1) * 128],
                                fb32[:, base + q * 128: base + (q + 1) * 128],
                                identf_sb[:],
                            )
                        ins.then_inc(petr, 1)
                    if u >= LAG:
                        ud = u - LAG
                        tensor.wait_ge(actcopy, ud + 1)
                        # bf16 view of the packed transposed pair:
                        # [p, q, b, par] with q = s*2+m, D = 256*m + 2*dp + par
                        vw = ftT[ud % TR][:].bitcast(BF16).rearrange(
                            "p (q b par) -> p q par b", q=4, par=2)
                        for s in range(2):
                            td = 2 * ud + s
                            i_d, j_d = td // SPB, td % SPB
                            for g in range(4):
                                m, par = g // 2, g % 2
                                ins = tensor.matmul(
                                    d_ps[i_d % DPR][:, 2 * j_d:2 * j_d + 2],
                                    vw[:, s * 2 + m, par, :],
                                    w_bfT_g[:, g, :],
                                    start=(g == 0), stop=(g == 3),
                                )
                        ins.then_inc(pedot, 1)

            @block.scalar
            def _(scalar):
                # --- W norms ---
                # pre must be waited at its FINAL value only (three DMAs inc
                # it; per-SDMA-engine skew makes intermediate thresholds
                # inexact and trips the semaphore-race checker)
                scalar.wait_ge(pre, 48)  # wrepT+w_f+lab ready
                for c in range(C):
                    scalar.activation(
                        out=sq_scr[c % 2][:], in_=w_f[:, c * D:(c + 1) * D],
                        func=AF.Square, accum_out=wss[:, c:c + 1],
                    )
                scalar.activation(out=wnorm[:], in_=wss[:], func=AF.Sqrt).then_inc(
                    sepi, 1
                )  # sepi=1
                scalar.wait_ge(vepi, 1)  # winv ready
                for c in range(C):
                    # w_bfT = wrepT * (1/||W_c||), cast to bf16.
                    # (tensor_scalar with an AP scalar = TensorScalarPtr
                    # mis-reads the scalar on this stack; Copy-with-scale on
                    # ScalarE is the validated path.)
                    scalar.activation(
                        out=w_bfT[:, c * 4:(c + 1) * 4],
                        in_=wrepT_sb[:, c * 4:(c + 1) * 4],
                        func=AF.Copy, scale=winv[:, c:c + 1],
                    )

                # --- main loop: PSUM->SBUF copies + sumsq share ---
                for i in range(NBLK):
                    for up in range(PPB):
                        u = i * PPB + up
                        scalar.wait_ge(petr, u + 1)
                        scalar.activation(
                            out=ftT[u % TR][:], in_=tpb[u % TR][:], func=AF.Copy
                        ).then_inc(actcopy, 1)
                    fb = fbs[i % NB]
                    for j in range(SQ_DVE, SPB):
                        t = i * SPB + j
                        ins = scalar.activation(
                            out=sq_scr[j % 2][:], in_=fb[:, j * SUB:(j + 1) * SUB],
                            func=AF.Square, accum_out=ss[:, t:t + 1],
                        )
                    ins.then_inc(ablk, 1)

                # --- epilogue (scalar part) ---
                scalar.wait_ge(ssdone, 1)  # DVE sumsq columns all written
                scalar.activation(out=norm[:], in_=ss[:], func=AF.Sqrt).then_inc(
                    sepi, 1
                )  # sepi=2
                scalar.wait_ge(vepi, 2)  # cos0 ready
                scalar.activation(out=sq2a[:], in_=cos0[:], func=AF.Square)
                scalar.activation(
                    out=sin0[:], in_=sq2a[:], func=AF.Sqrt, bias=1.0, scale=-1.0
                ).then_inc(sepi, 1)  # sepi=3
                scalar.wait_ge(vepi, 3)  # cos1 ready
                scalar.activation(out=sq2b[:], in_=cos1[:], func=AF.Square)
                scalar.activation(
                    out=sin1[:], in_=sq2b[:], func=AF.Sqrt, bias=1.0, scale=-1.0
                ).then_inc(sepi, 1)  # sepi=4

            @block.vector
            def _(vector):
                # --- preamble: winv + one-hot*64 vectors ---
                vector.wait_ge(sepi, 1)
                vector.reciprocal(winv[:], wnorm[:]).then_inc(vepi, 1)
                vector.wait_ge(pre, 48)  # lab_t ready
                vector.tensor_scalar(
                    oh64_0[:], lab_t[:], -S_SCALE, S_SCALE, OP.mult, OP.add
                )
                vector.tensor_scalar(oh64_1[:], lab_t[:], S_SCALE, None, OP.mult)

                # --- main loop: sumsq (gated on loads), then dot harvest ---
                for i in range(NBLK):
                    fb = fbs[i % NB]
                    for j in range(SQ_DVE):
                        t = i * SPB + j
                        sl = slice(j * SUB, (j + 1) * SUB)
                        if i == 0:
                            if j == 0:
                                vector.wait_ge(ld0x, 16)
                            elif j == 1:
                                vector.wait_ge(ld0[0], 16)
                            elif j in (2, 4):
                                vector.wait_ge(ld0[j // 2], 16)
                        elif i == NBLK - 1:
                            if j in (0, 2, 4):
                                vector.wait_ge(ld0[j // 2], 32)
                        elif j == 0:
                            vector.wait_ge(lds[i % NB], 16 * lds_thr(i))
                        ins = vector.scalar_tensor_tensor(
                            out=tt_scr[j % 2][:], in0=fb[:, sl], scalar=1.0,
                            in1=fb[:, sl], op0=OP.mult, op1=OP.mult,
                            accum_out=ss[:, t:t + 1],
                        )
                    if i == NBLK - 1:
                        ins.then_inc(ssdone, 1)
                    if i == 0:
                        # no harvest yet; vdone counts completed DVE iters
                        ins.then_inc(vdone, 1)
                    else:
                        # LAZY harvest: block i-1's dots finished long ago,
                        # so this pedot wait is free and DVE never stalls
                        # on the PE chain mid-kernel
                        vector.wait_ge(pedot, PPB * i)
                        vector.tensor_copy(
                            dd[:].rearrange("p (c t) -> p c t", c=2)
                                [:, :, SPB * (i - 1):SPB * i],
                            d_ps[(i - 1) % DPR][:].rearrange(
                                "p (j c) -> p c j", c=2),
                        ).then_inc(vdone, 1)
                if True:
                    # final harvest: the only wait on the tail of the PE chain
                    vector.wait_ge(pedot, PPB * NBLK)
                    vector.tensor_copy(
                        dd[:].rearrange("p (c t) -> p c t", c=2)
                            [:, :, SPB * (NBLK - 1):SPB * NBLK],
                        d_ps[(NBLK - 1) % DPR][:].rearrange(
                            "p (j c) -> p c j", c=2),
                    )

                # --- epilogue (vector part) ---
                vector.wait_ge(sepi, 2)  # norm ready
                vector.reciprocal(inv[:], norm[:])
                vector.tensor_tensor(cos0[:], dd[:, 0:T], inv[:], OP.mult)
                vector.tensor_scalar(
                    cos0[:], cos0[:], 1.0 - EPS, -1.0 + EPS, OP.min, OP.max
                ).then_inc(vepi, 1)  # vepi=2
                vector.tensor_tensor(cos1[:], dd[:, T:2 * T], inv[:], OP.mult)
                vector.tensor_scalar(
                    cos1[:], cos1[:], 1.0 - EPS, -1.0 + EPS, OP.min, OP.max
                ).then_inc(vepi, 1)  # vepi=3

                # out_c = 64*cos + oh64_c*(cos_m - cos)
                #       = 64*cos + oh64_c*((COS_M-1)*cos - SIN_M*sin)
                for c, (cosv, sinv, oh) in enumerate(
                    ((cos0, sin0, oh64_0), (cos1, sin1, oh64_1))
                ):
                    vector.wait_ge(sepi, 3 + c)  # sin_c ready
                    vector.tensor_scalar(
                        tmp1[:], cosv[:], COS_M - 1.0, None, OP.mult
                    )
                    vector.tensor_scalar(tmp2[:], sinv[:], SIN_M, None, OP.mult)
                    vector.tensor_tensor(tmp1[:], tmp1[:], tmp2[:], OP.subtract)
                    vector.tensor_tensor(tmp1[:], tmp1[:], oh[:], OP.mult)
                    vector.tensor_scalar(tmp2[:], cosv[:], S_SCALE, None, OP.mult)
                    vector.tensor_tensor(
                        out_stage[:, c * T:(c + 1) * T], tmp2[:], tmp1[:], OP.add
                    ).then_inc(vepi, 1)  # vepi=4 after c0, 5 after c1

    return nc


_NC = None


def _get_nc():
    global _NC
    if _NC is None:
        _NC = build_nc()
    return _NC


def _make_in_maps(feat, W, label):
    feat = np.ascontiguousarray(np.asarray(feat, dtype=np.float32))
    W = np.ascontiguousarray(np.asarray(W, dtype=np.float32))
    label = np.asarray(label)
    wr = np.ascontiguousarray(np.tile(W.reshape(1, C * D), (128, 1)))
    # wrepT[p, c*4 + m*2 + par] = W[c, 256*m + 2*p + par]
    # (pair-packed layout matching the f32-packed PE transposes)
    wrT = np.ascontiguousarray(
        W.reshape(C, 2, 128, 2).transpose(2, 0, 1, 3).reshape(128, 8)
    )
    ident = np.eye(128, dtype=np.float32)
    in_maps = []
    for core in range(NCORES):
        fs = feat[core * BS:(core + 1) * BS]
        ls = label[core * BS:(core + 1) * BS].astype(np.float32)
        # lab_dev[p, blk*8+j] = label[blk*1024 + p*8 + j]
        ls = ls.reshape(NBLK, 128, SPB).transpose(1, 0, 2).reshape(128, T)
        in_maps.append(
            {"feat": np.ascontiguousarray(fs), "wrep": wr, "wrepT": wrT,
             "lab": np.ascontiguousarray(ls), "identf": ident}
        )
    return in_maps


def _assemble(results):
    outs = []
    for core in range(NCORES):
        o = np.asarray(results[core]["out"])       # [128, C*T]
        o = o.reshape(128, C, NBLK, SPB)            # [p, c, blk, j]
        o = o.transpose(2, 0, 3, 1).reshape(BS, C)  # [blk, p, j, c]
        outs.append(o)
    return np.concatenate(outs, axis=0)


def run(feat, W, label, trace=False, **kw):
    nc = _get_nc()
    in_maps = _make_in_maps(feat, W, label)
    res = run_bass_kernel_spmd(
        nc, in_maps, core_ids=list(range(NCORES)), trace=trace, **kw
    )
    return _assemble(res.results), res


def kernel(feat, W, label):
    out, _ = run(feat, W, label, trace=False)
    return out


# revision 13
# speedup vs baseline: 1.0196x; 1.0196x over previous
"""ArcFace head forward on 8 Trainium2 NeuronCores (Bass, raw blocks).

Math (per batch row b, class c in {0,1}):
    feat_n = feat / max(||feat||, 1e-12)
    W_n    = W / max(||W_row||, 1e-12)
    cos    = clip(feat_n . W_n[c], -1+1e-7, 1-1e-7)
    cos_m  = cos*cos(0.5) - sqrt(1-cos^2)*sin(0.5)   # == cos(arccos(cos)+0.5)
    out    = 64 * (cos if c != label[b] else cos_m)

Distribution: pure data parallel: feat/label sharded along batch over 8
cores, W replicated; forward only, so no collectives.

Per-core pipeline (shard = 16384 rows x 512 f32 = 32 MB):
  - GpSimd SWDGE streams feat in 16 blocks of [128, 4096], converting
    f32 -> bf16 in flight (SDMA cast); 16 KB-contiguous per partition.
    First AND last blocks are split into chunk-DMAs so compute can
    start/finish at pair granularity instead of block granularity.
  - TensorE: per PAIR of [128,512] sub-tiles, 4x PE-transposes of the
    f32 VIEW of fb (each f32 element = 2 packed bf16 -> half the
    transpose instructions; transpose mode is bit-exact routing), then
    per sub-tile 4 accumulating matmuls: stationary = strided bf16
    views of the packed feat^T pair, moving = normalized-W^T pair
    chunks -> both class dots in PSUM [128,2]
  - ScalarE: copies transposed pairs PSUM->SBUF as f32 (matmul
    stationary must come from SBUF) + 2-of-8 sub-tiles' row
    sum-of-squares (activation Square + accum_out)
  - VectorE: 6-of-8 sub-tiles' sum-of-squares (fused self-mult STT,
    gated on the LOAD sem directly, not on ScalarE), per-block dot
    harvest from PSUM, and the batched epilogue
Ring-reuse safety: the loader double-waits (vdone AND ablk) before
recycling an fb slot, so VectorE never has to wait for ScalarE.
All dot accumulators share ONE PSUM bank (PE start_tensor_calc marks a
whole 2KB zero region but reads of marked-unwritten entries return the
stale (correct) data -- the quarter-tile transposes already rely on
this); that frees banks for a 4-deep transpose ring.
Row mapping: batch row b = blk*1024 + p*8 + j lives on partition p,
accumulator column t = blk*8 + j. Host glue only shards/reorders.
"""

import sys
from contextlib import ExitStack

import numpy as np

for _p in ("/opt/trn_rl_repo",):
    if _p not in sys.path:
        sys.path.insert(0, _p)

import concourse.bass as bass
import concourse.mybir as mybir
from concourse.bass_utils import run_bass_kernel_spmd

B, D, C = 131072, 512, 2
NCORES = 8
BS = B // NCORES          # 16384 rows per core
SUB = 512                 # columns per compute sub-tile
SPB = 8                   # sub-tiles per block
BLK_COLS = SUB * SPB      # 4096 (1024 batch rows)
NBLK = BS // (128 * SPB)  # 16
T = BS // 128             # 128 accumulator columns
NB = 7                    # fb ring depth (1 MB bf16 each)
SQ_DVE = 6                # sub-tiles per block whose sumsq runs on VectorE
PAIRS = T // 2            # transpose/copy granularity: 2 sub-tiles per pair
PPB = SPB // 2            # pairs per block (4)
TR = 3                    # tpb/ftT ring depth
LAG = 2                   # dots trail transposes by LAG pairs (needs TR >= LAG+1)
DPR = 5                   # d_ps ring (PSUM banks); decoupled from NB

S_SCALE = 64.0
MARGIN = 0.5
EPS = 1e-7
NORM_EPS = 1e-12
COS_M = float(np.cos(MARGIN))
SIN_M = float(np.sin(MARGIN))

F32 = mybir.dt.float32
BF16 = mybir.dt.bfloat16


def build_nc():
    nc = bass.Bass()
    AF = mybir.ActivationFunctionType
    OP = mybir.AluOpType

    feat = nc.declare_dram_parameter("feat", [BS, D], F32, isOutput=False)
    wrep = nc.declare_dram_parameter("wrep", [128, C * D], F32, isOutput=False)
    wrepT = nc.declare_dram_parameter("wrepT", [128, 2 * 4], F32, isOutput=False)
    lab = nc.declare_dram_parameter("lab", [128, T], F32, isOutput=False)
    identf = nc.declare_dram_parameter("identf", [128, 128], F32, isOutput=False)
    out = nc.declare_dram_parameter("out", [128, C * T], F32, isOutput=True)

    # feat[blk*1024 + p*8 + j, d] -> view[blk, p, j*512+d] (16KB/partition)
    feat_v = feat[:].rearrange("(blk p j) d -> blk p (j d)", p=128, j=SPB)

    with ExitStack() as ctx:
        def sb(name, shape, dt):
            return ctx.enter_context(nc.sbuf_tensor(name, shape, dt))

        def psum(name, shape, dt):
            return ctx.enter_context(nc.psum_tensor(name, shape, dt))

        def sem(name):
            return ctx.enter_context(nc.semaphore(name))

        w_f = sb("w_f", [128, C * D], F32)
        wrepT_sb = sb("wrepT_sb", [128, 8], F32)
        w_bfT = sb("w_bfT", [128, 8], BF16)     # [p, c*4+k] = WnT chunk layout
        lab_t = sb("lab_t", [128, T], F32)
        identf_sb = sb("identf_sb", [128, 128], F32)
        ss = sb("ss", [128, T], F32)
        dd = sb("dd", [128, C * T], F32)        # d0 | d1
        fbs = [sb(f"fb{k}", [128, BLK_COLS], BF16) for k in range(NB)]
        ftT = [sb(f"ftT{k}", [128, SUB], F32) for k in range(TR)]
        # scratch (dummy) outputs alternate buffers so consecutive same-engine
        # ops carry no WAW hazard (writes drain async; scratch is never read)
        sq_scr = [sb(f"sq_scr{k}", [128, SUB], F32) for k in range(2)]
        tt_scr = [sb(f"tt_scr{k}", [128, SUB], BF16) for k in range(2)]
        wss = sb("wss", [128, C], F32)
        wnorm = sb("wnorm", [128, C], F32)
        winv = sb("winv", [128, C], F32)
        norm = sb("norm", [128, T], F32)
        inv = sb("inv", [128, T], F32)
        cos0 = sb("cos0", [128, T], F32)
        cos1 = sb("cos1", [128, T], F32)
        sq2a = sb("sq2a", [128, T], F32)
        sq2b = sb("sq2b", [128, T], F32)
        sin0 = sb("sin0", [128, T], F32)
        sin1 = sb("sin1", [128, T], F32)
        tmp1 = sb("tmp1", [128, T], F32)
        tmp2 = sb("tmp2", [128, T], F32)
        oh64_0 = sb("oh64_0", [128, T], F32)    # 64*(1-lab)
        oh64_1 = sb("oh64_1", [128, T], F32)    # 64*lab
        out_stage = sb("out_stage", [128, C * T], F32)

        tpb = [psum(f"tpb{k}", [128, SUB], F32) for k in range(TR)]
        d_ps = [psum(f"dps{k}", [128, 2 * SPB], F32) for k in range(DPR)]

        pre = sem("pre")       # preamble DMAs: wrepT(16), w_f(32), lab(48)
        preI = sem("preI")     # identf DMA
        # One load-sem per fb ring slot: at most one DMA is ever outstanding
        # per sem (the ring gating guarantees it), so thresholds are exact
        # even under per-SDMA-engine completion skew.
        lds = [sem(f"ld{k}") for k in range(NB)]
        # blocks 0 and 15 are split into chunk-DMAs (one per pair) to hide
        # full-block latency at kernel start and end; ld0[c] hits 16 for
        # block 0's chunk c and 32 for block 15's chunk c.
        ld0 = [sem(f"ld0_{k}") for k in range(PPB)]
        ld0x = sem("ld0x")     # first 256KB half of block0 chunk0
        petr = sem("petr")     # PE transposes done for pair u -> u+1
        actcopy = sem("actcopy")  # ACT copy of pair u done -> u+1
        pedot = sem("pedot")   # PE dots done for pair u -> u+1
        ablk = sem("ablk")     # ACT done with block i -> i+1 (loader gating)
        vdone = sem("vdone")   # DVE done with block i -> i+1 (loader gating)
        ssdone = sem("ssdone")  # all DVE sumsq columns written
        sepi = sem("sepi")     # scalar milestones
        vepi = sem("vepi")     # vector milestones
        outd = sem("outd")

        # strided view of w_bfT: [p, g, c] with c-stride 4; g = qp*2+par
        w_bfT_g = w_bfT[:].rearrange("p (c g) -> p g c", g=4)

        # per-slot lds threshold for whole-block loads (blocks 1..14):
        # slot s receives incs from blocks s, s+NB, s+2NB (minus 0 and 15)
        def lds_thr(i):
            return i // NB + (1 if i % NB != 0 else 0)

        with nc.Block() as block:

            @block.sync
            def _(sync):
                sync.dma_start(out=identf_sb[:], in_=identf[:]).then_inc(preI, 16)
                sync.dma_start(out=wrepT_sb[:], in_=wrepT[:]).then_inc(pre, 16)
                sync.dma_start(out=w_f[:], in_=wrep[:]).then_inc(pre, 16)
                sync.dma_start(out=lab_t[:], in_=lab[:]).then_inc(pre, 16)
                sync.wait_ge(vepi, 4)
                sync.dma_start(
                    out=out[:, 0:T], in_=out_stage[:, 0:T]
                ).then_inc(outd, 16)
                sync.wait_ge(vepi, 5)
                sync.dma_start(
                    out=out[:, T:2 * T], in_=out_stage[:, T:2 * T]
                ).then_inc(outd, 16)
                sync.wait_ge(outd, 32)

            @block.gpsimd
            def _(gpsimd):
                gpsimd.dma_start(
                    out=fbs[0][:, 0:512], in_=feat_v[0][:, 0:512]
                ).then_inc(ld0x, 16)
                gpsimd.dma_start(
                    out=fbs[0][:, 512:1024], in_=feat_v[0][:, 512:1024]
                ).then_inc(ld0[0], 16)
                for c in range(1, PPB):
                    gpsimd.dma_start(
                        out=fbs[0][:, c * 1024:(c + 1) * 1024],
                        in_=feat_v[0][:, c * 1024:(c + 1) * 1024],
                    ).then_inc(ld0[c], 16)
                for i in range(1, NBLK):
                    if i >= NB:
                        # fb slot recycle: the tenant (block i-NB) must be
                        # fully consumed -- DVE sumsq (vdone counts completed
                        # DVE iters; iter k = STT(k) + lazy harvest(k-1)),
                        # ACT sumsq (ablk), and PE transposes (petr)
                        gpsimd.wait_ge(vdone, i - NB + 1)
                        gpsimd.wait_ge(ablk, i - NB + 1)
                        gpsimd.wait_ge(petr, PPB * (i - NB + 1))
                    if i == NBLK - 1:
                        # last block chunked so the tail drains at pair
                        # granularity (mirrors block 0)
                        for c in range(PPB):
                            gpsimd.dma_start(
                                out=fbs[i % NB][:, c * 1024:(c + 1) * 1024],
                                in_=feat_v[i][:, c * 1024:(c + 1) * 1024],
                            ).then_inc(ld0[c], 16)
                    else:
                        # converting DMA: f32 DRAM -> bf16 SBUF
                        gpsimd.dma_start(
                            out=fbs[i % NB][:], in_=feat_v[i]
                        ).then_inc(lds[i % NB], 16)

            @block.tensor
            def _(tensor):
                tensor.wait_ge(preI, 16)  # identf_sb ready
                # software-pipelined: transposes run LAG pairs ahead of dots.
                # Transposes operate on the f32 view of fb (bf16 pairs packed
                # per element): 4 transposes move a whole pair (2 sub-tiles).
                for u in range(PAIRS + LAG):
                    if u < PAIRS:
                        i = u // PPB
                        if i == 0:
                            if u == 0:
                                tensor.wait_ge(ld0x, 16)
                            else:
                                tensor.wait_ge(ld0[u], 16)
                        elif i == NBLK - 1:
                            tensor.wait_ge(ld0[u % PPB], 32)
                        elif u % PPB == 0:
                            tensor.wait_ge(lds[i % NB], 16 * lds_thr(i))
                        fb32 = fbs[i % NB][:].bitcast(F32)  # [128, 2048]
                        base = (u % PPB) * SUB              # f32 cols per pair
                        for q in range(4):
                            if u == 0 and q == 2:
                                tensor.wait_ge(ld0[0], 16)
                            ins = tensor.transpose(
                                tpb[u % TR][:, q * 128:(q + 1) * 128],
                                fb32[:, base + q * 128: base + (q + 1) * 128],
                                identf_sb[:],
                            )
                        ins.then_inc(petr, 1)
                    if u >= LAG:
                        ud = u - LAG
                        if ud % PPB == 0 and ud // PPB >= DPR:
                            # d_ps slot reuse: block ud//PPB - DPR was
                            # harvested in DVE iter (ud//PPB - DPR + 1);
                            # free in steady state, exact at the boundary
                            tensor.wait_ge(vdone, ud // PPB - DPR + 2)
                        tensor.wait_ge(actcopy, ud + 1)
                        # bf16 view of the packed transposed pair:
                        # [p, q, b, par] with q = s*2+m, D = 256*m + 2*dp + par
                        vw = ftT[ud % TR][:].bitcast(BF16).rearrange(
                            "p (q b par) -> p q par b", q=4, par=2)
                        for s in range(2):
                            td = 2 * ud + s
                            i_d, j_d = td // SPB, td % SPB
                            for g in range(4):
                                m, par = g // 2, g % 2
                                ins = tensor.matmul(
                                    d_ps[i_d % DPR][:, 2 * j_d:2 * j_d + 2],
                                    vw[:, s * 2 + m, par, :],
                                    w_bfT_g[:, g, :],
                                    start=(g == 0), stop=(g == 3),
                                )
                        ins.then_inc(pedot, 1)

            @block.scalar
            def _(scalar):
                # --- W norms ---
                # pre must be waited at its FINAL value only (three DMAs inc
                # it; per-SDMA-engine skew makes intermediate thresholds
                # inexact and trips the semaphore-race checker)
                scalar.wait_ge(pre, 48)  # wrepT+w_f+lab ready
                for c in range(C):
                    scalar.activation(
                        out=sq_scr[c % 2][:], in_=w_f[:, c * D:(c + 1) * D],
                        func=AF.Square, accum_out=wss[:, c:c + 1],
                    )
                scalar.activation(out=wnorm[:], in_=wss[:], func=AF.Sqrt).then_inc(
                    sepi, 1
                )  # sepi=1
                scalar.wait_ge(vepi, 1)  # winv ready
                for c in range(C):
                    # w_bfT = wrepT * (1/||W_c||), cast to bf16.
                    # (tensor_scalar with an AP scalar = TensorScalarPtr
                    # mis-reads the scalar on this stack; Copy-with-scale on
                    # ScalarE is the validated path.)
                    scalar.activation(
                        out=w_bfT[:, c * 4:(c + 1) * 4],
                        in_=wrepT_sb[:, c * 4:(c + 1) * 4],
                        func=AF.Copy, scale=winv[:, c:c + 1],
                    )

                # --- main loop: PSUM->SBUF copies + sumsq share ---
                for i in range(NBLK):
                    for up in range(PPB):
                        u = i * PPB + up
                        scalar.wait_ge(petr, u + 1)
                        scalar.activation(
                            out=ftT[u % TR][:], in_=tpb[u % TR][:], func=AF.Copy
                        ).then_inc(actcopy, 1)
                    fb = fbs[i % NB]
                    for j in range(SQ_DVE, SPB):
                        t = i * SPB + j
                        ins = scalar.activation(
                            out=sq_scr[j % 2][:], in_=fb[:, j * SUB:(j + 1) * SUB],
                            func=AF.Square, accum_out=ss[:, t:t + 1],
                        )
                    ins.then_inc(ablk, 1)

                # --- epilogue (scalar part) ---
                scalar.wait_ge(ssdone, 1)  # DVE sumsq columns all written
                scalar.activation(out=norm[:], in_=ss[:], func=AF.Sqrt).then_inc(
                    sepi, 1
                )  # sepi=2
                scalar.wait_ge(vepi, 2)  # cos0 ready
                scalar.activation(out=sq2a[:], in_=cos0[:], func=AF.Square)
                scalar.activation(
                    out=sin0[:], in_=sq2a[:], func=AF.Sqrt, bias=1.0, scale=-1.0
                ).then_inc(sepi, 1)  # sepi=3
                scalar.wait_ge(vepi, 3)  # cos1 ready
                scalar.activation(out=sq2b[:], in_=cos1[:], func=AF.Square)
                scalar.activation(
                    out=sin1[:], in_=sq2b[:], func=AF.Sqrt, bias=1.0, scale=-1.0
                ).then_inc(sepi, 1)  # sepi=4

            @block.vector
            def _(vector):
                # --- preamble: winv + one-hot*64 vectors ---
                vector.wait_ge(sepi, 1)
                vector.reciprocal(winv[:], wnorm[:]).then_inc(vepi, 1)
                vector.wait_ge(pre, 48)  # lab_t ready
                vector.tensor_scalar(
                    oh64_0[:], lab_t[:], -S_SCALE, S_SCALE, OP.mult, OP.add
                )
                vector.tensor_scalar(oh64_1[:], lab_t[:], S_SCALE, None, OP.mult)

                # --- main loop: sumsq (gated on loads), then dot harvest ---
                for i in range(NBLK):
                    fb = fbs[i % NB]
                    for j in range(SQ_DVE):
                        t = i * SPB + j
                        sl = slice(j * SUB, (j + 1) * SUB)
                        if i == 0:
                            if j == 0:
                                vector.wait_ge(ld0x, 16)
                            elif j == 1:
                                vector.wait_ge(ld0[0], 16)
                            elif j in (2, 4):
                                vector.wait_ge(ld0[j // 2], 16)
                        elif i == NBLK - 1:
                            if j in (0, 2, 4):
                                vector.wait_ge(ld0[j // 2], 32)
                        elif j == 0:
                            vector.wait_ge(lds[i % NB], 16 * lds_thr(i))
                        ins = vector.scalar_tensor_tensor(
                            out=tt_scr[j % 2][:], in0=fb[:, sl], scalar=1.0,
                            in1=fb[:, sl], op0=OP.mult, op1=OP.mult,
                            accum_out=ss[:, t:t + 1],
                        )
                    if i == NBLK - 1:
                        ins.then_inc(ssdone, 1)
                    if i == 0:
                        # no harvest yet; vdone counts completed DVE iters
                        ins.then_inc(vdone, 1)
                    else:
                        # LAZY harvest: block i-1's dots finished long ago,
                        # so this pedot wait is free and DVE never stalls
                        # on the PE chain mid-kernel
                        vector.wait_ge(pedot, PPB * i)
                        vector.tensor_copy(
                            dd[:].rearrange("p (c t) -> p c t", c=2)
                                [:, :, SPB * (i - 1):SPB * i],
                            d_ps[(i - 1) % DPR][:].rearrange(
                                "p (j c) -> p c j", c=2),
                        ).then_inc(vdone, 1)
                if True:
                    # final harvest: the only wait on the tail of the PE chain
                    vector.wait_ge(pedot, PPB * NBLK)
                    vector.tensor_copy(
                        dd[:].rearrange("p (c t) -> p c t", c=2)
                            [:, :, SPB * (NBLK - 1):SPB * NBLK],
                        d_ps[(NBLK - 1) % DPR][:].rearrange(
                            "p (j c) -> p c j", c=2),
                    )

                # --- epilogue (vector part) ---
                vector.wait_ge(sepi, 2)  # norm ready
                vector.reciprocal(inv[:], norm[:])
                vector.tensor_tensor(cos0[:], dd[:, 0:T], inv[:], OP.mult)
                vector.tensor_scalar(
                    cos0[:], cos0[:], 1.0 - EPS, -1.0 + EPS, OP.min, OP.max
                ).then_inc(vepi, 1)  # vepi=2
                vector.tensor_tensor(cos1[:], dd[:, T:2 * T], inv[:], OP.mult)
                vector.tensor_scalar(
                    cos1[:], cos1[:], 1.0 - EPS, -1.0 + EPS, OP.min, OP.max
                ).then_inc(vepi, 1)  # vepi=3

                # out_c = 64*cos + oh64_c*(cos_m - cos)
                #       = 64*cos + oh64_c*((COS_M-1)*cos - SIN_M*sin)
                for c, (cosv, sinv, oh) in enumerate(
                    ((cos0, sin0, oh64_0), (cos1, sin1, oh64_1))
                ):
                    vector.wait_ge(sepi, 3 + c)  # sin_c ready
                    vector.tensor_scalar(
                        tmp1[:], cosv[:], COS_M - 1.0, None, OP.mult
                    )
                    vector.tensor_scalar(tmp2[:], sinv[:], SIN_M, None, OP.mult)
                    vector.tensor_tensor(tmp1[:], tmp1[:], tmp2[:], OP.subtract)
                    vector.tensor_tensor(tmp1[:], tmp1[:], oh[:], OP.mult)
                    vector.tensor_scalar(tmp2[:], cosv[:], S_SCALE, None, OP.mult)
                    vector.tensor_tensor(
                        out_stage[:, c * T:(c + 1) * T], tmp2[:], tmp1[:], OP.add
                    ).then_inc(vepi, 1)  # vepi=4 after c0, 5 after c1

    return nc


_NC = None


def _get_nc():
    global _NC
    if _NC is None:
        _NC = build_nc()
    return _NC


def _make_in_maps(feat, W, label):
    feat = np.ascontiguousarray(np.asarray(feat, dtype=np.float32))
    W = np.ascontiguousarray(np.asarray(W, dtype=np.float32))
    label = np.asarray(label)
    wr = np.ascontiguousarray(np.tile(W.reshape(1, C * D), (128, 1)))
    # wrepT[p, c*4 + m*2 + par] = W[c, 256*m + 2*p + par]
    # (pair-packed layout matching the f32-packed PE transposes)
    wrT = np.ascontiguousarray(
        W.reshape(C, 2, 128, 2).transpose(2, 0, 1, 3).reshape(128, 8)
    )
    ident = np.eye(128, dtype=np.float32)
    in_maps = []
    for core in range(NCORES):
        fs = feat[core * BS:(core + 1) * BS]
        ls = label[core * BS:(core + 1) * BS].astype(np.float32)
        # lab_dev[p, blk*8+j] = label[blk*1024 + p*8 + j]
        ls = ls.reshape(NBLK, 128, SPB).transpose(1, 0, 2).reshape(128, T)
        in_maps.append(
            {"feat": np.ascontiguousarray(fs), "wrep": wr, "wrepT": wrT,
             "lab": np.ascontiguousarray(ls), "identf": ident}
        )
    return in_maps


def _assemble(results):
    outs = []
    for core in range(NCORES):
        o = np.asarray(results[core]["out"])       # [128, C*T]
        o = o.reshape(128, C, NBLK, SPB)            # [p, c, blk, j]
        o = o.transpose(2, 0, 3, 1).reshape(BS, C)  # [blk, p, j, c]
        outs.append(o)
    return np.concatenate(outs, axis=0)


def run(feat, W, label, trace=False, **kw):
    nc = _get_nc()
    in_maps = _make_in_maps(feat, W, label)
    res = run_bass_kernel_spmd(
        nc, in_maps, core_ids=list(range(NCORES)), trace=trace, **kw
    )
    return _assemble(res.results), res


def kernel(feat, W, label):
    out, _ = run(feat, W, label, trace=False)
    return out
